# revision 9
# baseline (speedup 1.0000x reference)
"""Trainium2 Bass kernel for nn_LocalConv2DLayer (fuzzy local conv membership layer).

Math: for input x[B,C,H,W], bounds l_o < r_o forming 32 uniform bins over
[-1,1], the reference computes, per output pixel (b,o,i,j):

    res = sum_{c,kh,kw} (relu(clip(p-l,-1,1)) * relu(clip(r-p,-1,1)) * 4/(r-l)^2)^2

with p = x[b,c,i+kh,j+kw]. Each pixel value falls in exactly one bin; its
contribution to that bin is 16*(f*(1-f))^2 with f = frac((p-l_0)/width),
and zero to every other bin.

Kernel structure per core (2 batches, SPMD over 8 cores):
  - layout: partitions = (b_local, h) = 128, free = (c, w) = 192
  - prep (DVE): z = scale*x + bias (bias shifted +64 so z>0 in practice),
    f = z mod 1, idx = z - f, idxlo = idx mod 4, dhi = idx - idxlo
  - val chain on ScalarE: fm2 = Square(f - 0.5), val = Square(-128*fm2+32)
    (= 2^10*(4f(1-f))^2; band carries the 2^-10 compensation)
  - masks (DVE): ehi = [dhi == 64+4g] vs iota consts, e_lo = [idxlo == l],
    vlo = e_lo * val; per 8-channel block msq = ehi * vlo
  - per block: banded matmul on PE sums kh-window + channel sum via PSUM
    accumulation; ScalarE copies PSUM->SBUF fp16
  - tail (DVE): per 16-channel pair, horizontal 5-tap via 3 shifted adds
  - out DMAs split across sync + gpsimd queues (fp16 -> fp32 cast in DMA)
  - PE warm-up matmuls on a memset scratch tile run during the startup
    window so real matmuls hit full clock.
"""

import numpy as np

B, C, O, H, W = 16, 3, 32, 64, 64
KS = 5
NH, NW = H - KS + 1, W - KS + 1  # 60, 60
NCORES = 8
BPC = B // NCORES  # batches per core
P = BPC * H        # 128 partitions = (b_local, h)
M = BPC * NH       # 120 matmul output rows = (b_local, i)
OB = 8             # output channels per block
NBLK = O // OB     # 4
NLO, NHI = 4, O // 4
FD = C * W         # 192
N_WARM = 22        # PE ramp warm-up matmuls (256 cols each)
POOL_MSQ = (1,)    # blocks whose msq product runs on gpsimd instead of DVE

_CACHE = {}


def _build(scale: float, bias: float):
    import concourse.bass as bass
    import concourse.tile as tile
    from concourse import mybir

    dt = mybir.dt
    Alu = mybir.AluOpType
    Act = mybir.ActivationFunctionType

    nc = bass.Bass(enable_partition_id=False)
    x_d = nc.declare_dram_parameter("xb", [P, FD], dt.float32, isOutput=False)
    b2_d = nc.declare_dram_parameter("blob2", [P, M // 2], dt.float32, isOutput=False)
    out_d = nc.declare_dram_parameter("out", [M, O, NW], dt.float32, isOutput=True)

    with tile.TileContext(nc) as tc:
        with (
            tc.tile_pool(name="s", bufs=1) as s,
            tc.tile_pool(name="ps", bufs=1, space="PSUM") as ps,
        ):
            # input DMAs first: x on sync queue, band on scalar queue
            x_sb = s.tile([P, FD], dt.float32)
            nc.sync.dma_start(out=x_sb, in_=x_d[:])
            b2_sb = s.tile([P, M // 2], dt.float32)
            nc.scalar.dma_start(out=b2_sb, in_=b2_d[:])
            band_sb = b2_sb.bitcast(dt.float16)  # [P, M], 2^-10 band

            # startup-window generated constants + warmup feed
            warm = s.tile([P, 256], dt.float16)
            nc.gpsimd.memset(warm, 0.0)
            c_lo = s.tile([P, NLO, FD], dt.float16)
            nc.gpsimd.iota(c_lo, [[1, NLO], [0, FD]], base=0,
                           channel_multiplier=0,
                           allow_small_or_imprecise_dtypes=True)
            c_hi = s.tile([P, NHI, FD], dt.float16)
            nc.gpsimd.iota(c_hi, [[1, NHI], [0, FD]], base=16,
                           channel_multiplier=0,
                           allow_small_or_imprecise_dtypes=True)

            # per-partition bias constants for the Square activations
            # (non-Copy activation bias must be an SBUF AP)
            b0 = s.tile([P, 1], dt.float32)
            nc.gpsimd.memset(b0, 0.0)
            b32 = s.tile([P, 1], dt.float32)
            nc.gpsimd.memset(b32, 32.0)

            # preload both ScalarE act tables off the critical path: a tiny
            # Copy before the first Square forces Copy-set then Square-set
            # loads during the startup window.
            dummy = s.tile([P, 1], dt.float16)
            nc.scalar.copy(dummy, warm[:, 0:1])

            # PE ramp warm-up on memset scratch
            warm_ps = ps.tile([M, 256], dt.float32)
            for _ in range(N_WARM):
                nc.tensor.matmul(warm_ps, lhsT=warm[:, 0:M], rhs=warm,
                                 start=True, stop=True)

            # ---- DVE prep (MAGIC rounding; DVE has no mod ALU) ----
            MAGIC = 12582912.0  # 1.5 * 2^23; x+M-M == rne(x) for |x| < 2^22
            # z2 = z - 0.5; rne(z2) = floor(z) (bin-edge ties have val==0)
            z2 = s.tile([P, FD], dt.float32)
            nc.vector.tensor_scalar(z2, x_sb, float(scale), float(bias) - 0.5,
                                    op0=Alu.mult, op1=Alu.add)
            t_mag = s.tile([P, FD], dt.float32)
            nc.vector.tensor_scalar(t_mag, z2, MAGIC, None, op0=Alu.add)
            idx = s.tile([P, FD], dt.float32)
            nc.vector.tensor_scalar(idx, t_mag, MAGIC, None, op0=Alu.subtract)
            fm = s.tile([P, FD], dt.float32)
            nc.vector.tensor_sub(fm, z2, idx)
            # u = z/4 - 0.5; rne(u) = floor(z/4) (ties land on integer z -> val==0)
            u = s.tile([P, FD], dt.float32)
            nc.vector.tensor_scalar(u, z2, 0.25, 0.375, op0=Alu.mult, op1=Alu.subtract)
            ihi = s.tile([P, FD], dt.float16)
            nc.vector.tensor_scalar(ihi, u, MAGIC, MAGIC, op0=Alu.add, op1=Alu.subtract)
            hi4 = s.tile([P, FD], dt.float32)
            nc.vector.tensor_scalar(hi4, ihi, 4.0, None, op0=Alu.mult)
            idxlo = s.tile([P, FD], dt.float16)
            nc.vector.tensor_sub(idxlo, idx, hi4)

            # ---- ScalarE val chain: val = (32 - 128*fm^2)^2 = 2^10*(4f(1-f))^2
            fm2 = s.tile([P, FD], dt.float32)
            nc.scalar.activation(fm2, fm, Act.Square, bias=b0, scale=1.0)
            val = s.tile([P, FD], dt.float16)
            nc.scalar.activation(val, fm2, Act.Square, bias=b32, scale=-128.0)

            # ---- masks ----
            ehi = s.tile([P, NHI, FD], dt.float16)
            ihi_b = ihi.rearrange("p (g f) -> p g f", g=1).broadcast_to([P, NHI, FD])
            nc.vector.tensor_tensor(ehi, ihi_b, c_hi, Alu.is_equal)
            e_lo = s.tile([P, NLO, FD], dt.float16)
            idxlo_b = idxlo.rearrange("p (l f) -> p l f", l=1).broadcast_to([P, NLO, FD])
            nc.vector.tensor_tensor(e_lo, idxlo_b, c_lo, Alu.is_equal)
            vlo = s.tile([P, NLO, FD], dt.float16)
            val_b = val.rearrange("p (l f) -> p l f", l=1).broadcast_to([P, NLO, FD])
            nc.vector.tensor_mul(vlo, e_lo, val_b)

            msq = s.tile([P, NHI, NLO, FD], dt.float16)
            vlo_b = vlo.rearrange("p (g l) f -> p g l f", g=1).broadcast_to([P, 2, NLO, FD])
            for b in range(NBLK):
                eng = nc.gpsimd if b in POOL_MSQ else nc.vector
                ehi_blk = (
                    ehi[:, 2 * b : 2 * b + 2, :]
                    .rearrange("p g (l f) -> p g l f", l=1)
                    .broadcast_to([P, 2, NLO, FD])
                )
                eng.tensor_mul(msq[:, 2 * b : 2 * b + 2], ehi_blk, vlo_b)

            # ---- per-block matmul + PSUM->SBUF copy ----
            v_sb = s.tile([M, NBLK, OB, W], dt.float16)
            for b in range(NBLK):
                vps = ps.tile([M, OB, W], dt.float32, name=f"vps{b}")
                rhs = msq[:, 2 * b : 2 * b + 2].rearrange(
                    "p g l (c w) -> p (g l) c w", c=C)
                for c in range(C):
                    nc.tensor.matmul(vps, lhsT=band_sb, rhs=rhs[:, :, c, :],
                                     start=(c == 0), stop=(c == C - 1))
                nc.scalar.copy(v_sb[:, b], vps)

            # ---- tail: horizontal 5-tap per 16-channel pair + out DMAs ----
            res_all = s.tile([M, O, NW], dt.float16)
            Epair = s.tile([M, 2, 2 * OB, W - 1], dt.float16)
            Tpair = s.tile([M, 2, 2 * OB, NW], dt.float16)
            for pr in range(2):
                vp = v_sb[:, 2 * pr : 2 * pr + 2].rearrange("m b o w -> m (b o) w")
                E = Epair[:, pr]
                nc.vector.tensor_add(E, vp[:, :, 0 : W - 1], vp[:, :, 1:W])
                T = Tpair[:, pr]
                nc.vector.tensor_add(T, E[:, :, 0:NW], E[:, :, 2 : NW + 2])
                res = res_all[:, pr * 2 * OB : (pr + 1) * 2 * OB]
                nc.vector.tensor_add(res, T, vp[:, :, 4 : 4 + NW])
                # casting DMAs (fp16 -> fp32) are SWDGE/gpsimd-only
                lo_ch = pr * 2 * OB
                nc.gpsimd.dma_start(out=out_d[:, lo_ch : lo_ch + OB],
                                    in_=res_all[:, lo_ch : lo_ch + OB])
                nc.gpsimd.dma_start(out=out_d[:, lo_ch + OB : lo_ch + 2 * OB],
                                    in_=res_all[:, lo_ch + OB : lo_ch + 2 * OB])
    return nc


def _legalize_multiwaits(bir_json_bytes):
    """Split multi-wait instructions into standalone EventSemaphore waits.

    The walrus codegen in this toolchain accepts at most one inline sync
    wait per compute-engine instruction ("Too many sync wait commands").
    Tile emits joins with several waits; moving the extras onto
    EventSemaphore instructions issued immediately before, on the same
    engine queue, is semantically identical.
    """
    import json

    j = json.loads(bir_json_bytes)
    for fn in j["functions"]:
        for blk in fn["blocks"]:
            new_insts = []
            for inst in blk["instructions"]:
                si = inst.get("sync_info") or {}
                waits = si.get("on_wait") or []
                if len(waits) > 1:
                    for k, w in enumerate(waits[:-1]):
                        new_insts.append(
                            {
                                "debug": inst.get("debug"),
                                "engine": inst["engine"],
                                "ins": [],
                                "name": f"{inst['name']}_syncw{k}",
                                "opcode": "EventSemaphore",
                                "outs": [],
                                "sync_info": {"on_update": [], "on_wait": [w]},
                            }
                        )
                    si["on_wait"] = [waits[-1]]
                new_insts.append(inst)
            blk["instructions"] = new_insts
    return json.dumps(j).encode()


def _band_np():
    band = np.zeros((P, M), np.float16)
    for b in range(BPC):
        for h in range(H):
            for i in range(NH):
                if 0 <= h - i < KS:
                    band[b * H + h, b * NH + i] = 2.0 ** -10
    return band


def _get_built(scale, bias):
    key = (round(float(scale), 9), round(float(bias), 9))
    if key not in _CACHE:
        nc = _build(float(scale), float(bias))
        legal = _legalize_multiwaits(nc.to_json_bytes())
        nc.to_json_bytes = lambda: legal
        _CACHE[key] = nc
    return _CACHE[key]


def kernel(x, left_bounds, right_bounds):
    x = np.ascontiguousarray(x, np.float32)
    lb = np.asarray(left_bounds, np.float32).reshape(O, -1)
    rb = np.asarray(right_bounds, np.float32).reshape(O, -1)
    widths = rb[:, 0] - lb[:, 0]
    width = float(widths[0])
    # the kernel's bin decomposition requires uniform contiguous bins
    assert np.allclose(widths, width, rtol=1e-5), "non-uniform bounds unsupported"
    assert np.allclose(lb[1:, 0], rb[:-1, 0], atol=1e-6), "bins must tile the domain"
    scale = 1.0 / width
    # +64 shift keeps z positive for |x| within ~5 sigma so trunc-mod == floor-mod;
    # bin o lives at idx == 64 + o, matching the iota compare constants.
    bias = -float(lb[0, 0]) * scale + 64.0

    nc = _get_built(scale, bias)
    band = _band_np()
    band_f32view = np.ascontiguousarray(band).view(np.float32)  # [P, M//2]
    in_maps = []
    for k in range(NCORES):
        xc = x[BPC * k : BPC * (k + 1)]  # [BPC, C, H, W]
        xt = np.ascontiguousarray(xc.transpose(0, 2, 1, 3).reshape(P, C * W))
        in_maps.append({"xb": xt, "blob2": band_f32view})

    from concourse.bass_utils import run_bass_kernel_spmd

    r = run_bass_kernel_spmd(nc, in_maps, list(range(NCORES)))
    global _LAST_RESULT
    _LAST_RESULT = r
    parts = []
    for k in range(NCORES):
        oc = r.results[k]["out"]  # [M, O, NW] = [(b i), o, j]
        oc = oc.reshape(BPC, NH, O, NW).transpose(0, 2, 1, 3)
        parts.append(np.ascontiguousarray(oc))
    out = np.concatenate(parts, axis=0)
    return np.ascontiguousarray(out, np.float32)


_LAST_RESULT = None


# revision 16
# speedup vs baseline: 1.1345x; 1.1345x over previous
"""Trainium2 Bass kernel for nn_LocalConv2DLayer (fuzzy local conv membership layer).

Math: for input x[B,C,H,W], bounds l_o < r_o forming 32 uniform bins over
[-1,1], the reference computes, per output pixel (b,o,i,j):

    res = sum_{c,kh,kw} (relu(clip(p-l,-1,1)) * relu(clip(r-p,-1,1)) * 4/(r-l)^2)^2

with p = x[b,c,i+kh,j+kw]. Each pixel value falls in exactly one bin; its
contribution to that bin is 16*(f*(1-f))^2 with f = frac((p-l_0)/width),
and zero to every other bin.

Kernel structure per core (2 batches, SPMD over 8 cores):
  - layout: partitions = (b_local, h) = 128, free = (c, w) = 192
  - prep (DVE): z = scale*x + bias (bias shifted +64 so z>0 in practice),
    f = z mod 1, idx = z - f, idxlo = idx mod 4, dhi = idx - idxlo
  - val chain on ScalarE: fm2 = Square(f - 0.5), val = Square(-128*fm2+32)
    (= 2^10*(4f(1-f))^2; band carries the 2^-10 compensation)
  - masks (DVE): ehi = [dhi == 64+4g] vs iota consts, e_lo = [idxlo == l],
    vlo = e_lo * val; per 8-channel block msq = ehi * vlo
  - per block: banded matmul on PE sums kh-window + channel sum via PSUM
    accumulation; ScalarE copies PSUM->SBUF fp16
  - tail (DVE): per 16-channel pair, horizontal 5-tap via 3 shifted adds
  - out DMAs split across sync + gpsimd queues (fp16 -> fp32 cast in DMA)
  - PE warm-up matmuls on a memset scratch tile run during the startup
    window so real matmuls hit full clock.
"""

import numpy as np

B, C, O, H, W = 16, 3, 32, 64, 64
KS = 5
NH, NW = H - KS + 1, W - KS + 1  # 60, 60
NCORES = 8
BPC = B // NCORES  # batches per core
P = BPC * H        # 128 partitions = (b_local, h)
M = BPC * NH       # 120 matmul output rows = (b_local, i)
OB = 8             # output channels per block
NBLK = O // OB     # 4
NLO, NHI = 4, O // 4
FD = C * W         # 192
N_WARM = 30        # PE ramp warm-up matmuls (512 cols each)
POOL_MSQ = ()      # gpsimd elementwise contends with DVE on the shared SBUF
                   # port (measured 4x slowdown on BOTH engines) - keep empty

_CACHE = {}


def _build(scale: float, bias: float):
    import concourse.bass as bass
    import concourse.tile as tile
    from concourse import mybir

    dt = mybir.dt
    Alu = mybir.AluOpType
    Act = mybir.ActivationFunctionType

    nc = bass.Bass(enable_partition_id=False)
    x_d = nc.declare_dram_parameter("xb", [P, FD], dt.float32, isOutput=False)
    # blob2 per partition (fp16): band[M] + c_lo[4*FD] + c_hi[8*FD]
    B2H = M + NLO * FD + NHI * FD
    b2_d = nc.declare_dram_parameter("blob2", [P, B2H // 2], dt.float32, isOutput=False)
    out_d = nc.declare_dram_parameter("out", [M, O, NW], dt.float32, isOutput=True)

    with tile.TileContext(nc) as tc:
        with (
            tc.tile_pool(name="s", bufs=1) as s,
            tc.tile_pool(name="ps", bufs=1, space="PSUM") as ps,
        ):
            # input DMAs first: x on sync queue, band on scalar queue
            x_sb = s.tile([P, FD], dt.float32)
            nc.sync.dma_start(out=x_sb, in_=x_d[:])
            b2_sb = s.tile([P, B2H // 2], dt.float32)
            nc.scalar.dma_start(out=b2_sb, in_=b2_d[:])
            b2h = b2_sb.bitcast(dt.float16)  # [P, B2H]
            band_sb = b2h[:, 0:M]                         # 2^-10 band
            c_lo = b2h[:, M : M + NLO * FD].rearrange("p (l f) -> p l f", l=NLO)
            c_hi = b2h[:, M + NLO * FD :].rearrange("p (g f) -> p g f", g=NHI)

            # warmup feed (gpsimd memset finishes long before DVE starts,
            # so no shared-port contention)
            warm = s.tile([P, 512], dt.float16)
            nc.gpsimd.memset(warm, 0.0)

            # per-partition bias constants for the Square activations
            # (non-Copy activation bias must be an SBUF AP)
            b0 = s.tile([P, 1], dt.float32)
            nc.gpsimd.memset(b0, 0.0)
            b32 = s.tile([P, 1], dt.float32)
            nc.gpsimd.memset(b32, 32.0)

            # preload both ScalarE act tables off the critical path: a tiny
            # Copy before the first Square forces Copy-set then Square-set
            # loads during the startup window.
            dummy = s.tile([P, 1], dt.float16)
            nc.scalar.copy(dummy, warm[:, 0:1])

            # PE ramp warm-up on memset scratch
            warm_ps = ps.tile([M, 512], dt.float32)
            for _ in range(N_WARM):
                nc.tensor.matmul(warm_ps, lhsT=warm[:, 0:M], rhs=warm,
                                 start=True, stop=True)

            # ---- DVE prep (MAGIC rounding; DVE has no mod ALU) ----
            MAGIC = 12582912.0  # 1.5 * 2^23; x+M-M == rne(x) for |x| < 2^22
            # z2 = z - 0.5; rne(z2) = floor(z) (bin-edge ties have val==0)
            z2 = s.tile([P, FD], dt.float32)
            nc.vector.tensor_scalar(z2, x_sb, float(scale), float(bias) - 0.5,
                                    op0=Alu.mult, op1=Alu.add)
            t_mag = s.tile([P, FD], dt.float32)
            nc.vector.tensor_scalar(t_mag, z2, MAGIC, None, op0=Alu.add)
            idx = s.tile([P, FD], dt.float32)
            nc.vector.tensor_scalar(idx, t_mag, MAGIC, None, op0=Alu.subtract)
            fm = s.tile([P, FD], dt.float32)
            nc.vector.tensor_sub(fm, z2, idx)
            # u = z/4 - 0.5; rne(u) = floor(z/4) (ties land on integer z -> val==0)
            u = s.tile([P, FD], dt.float32)
            nc.vector.tensor_scalar(u, z2, 0.25, 0.375, op0=Alu.mult, op1=Alu.subtract)
            ihi = s.tile([P, FD], dt.float16)
            nc.vector.tensor_scalar(ihi, u, MAGIC, MAGIC, op0=Alu.add, op1=Alu.subtract)
            idxlo = s.tile([P, FD], dt.float16)
            # idxlo = idx - 4*ihi, fused: (ihi * -4) + idx
            nc.vector.scalar_tensor_tensor(idxlo, ihi, -4.0, idx,
                                           op0=Alu.mult, op1=Alu.add)

            # ---- ScalarE val chain: val = (32 - 128*fm^2)^2 = 2^10*(4f(1-f))^2
            fm2 = s.tile([P, FD], dt.float32)
            nc.scalar.activation(fm2, fm, Act.Square, bias=b0, scale=1.0)
            val = s.tile([P, FD], dt.float16)
            nc.scalar.activation(val, fm2, Act.Square, bias=b32, scale=-128.0)

            # ---- masks ----
            ehi = s.tile([P, NHI, FD], dt.float16)
            ihi_b = ihi.rearrange("p (g f) -> p g f", g=1).broadcast_to([P, NHI, FD])
            nc.vector.tensor_tensor(ehi, ihi_b, c_hi, Alu.is_equal)
            e_lo = s.tile([P, NLO, FD], dt.float16)
            idxlo_b = idxlo.rearrange("p (l f) -> p l f", l=1).broadcast_to([P, NLO, FD])
            nc.vector.tensor_tensor(e_lo, idxlo_b, c_lo, Alu.is_equal)
            vlo = s.tile([P, NLO, FD], dt.float16)
            val_b = val.rearrange("p (l f) -> p l f", l=1).broadcast_to([P, NLO, FD])
            nc.vector.tensor_mul(vlo, e_lo, val_b)

            msq = s.tile([P, NHI, NLO, FD], dt.float16)
            vlo_b = vlo.rearrange("p (g l) f -> p g l f", g=1).broadcast_to([P, 2, NLO, FD])
            for b in range(NBLK):
                eng = nc.gpsimd if b in POOL_MSQ else nc.vector
                ehi_blk = (
                    ehi[:, 2 * b : 2 * b + 2, :]
                    .rearrange("p g (l f) -> p g l f", l=1)
                    .broadcast_to([P, 2, NLO, FD])
                )
                eng.tensor_mul(msq[:, 2 * b : 2 * b + 2], ehi_blk, vlo_b)

            # ---- per-block matmul + PSUM->SBUF copy ----
            v_sb = s.tile([M, NBLK, OB, W], dt.float16)
            for b in range(NBLK):
                vps = ps.tile([M, OB, W], dt.float32, name=f"vps{b}")
                rhs = msq[:, 2 * b : 2 * b + 2].rearrange(
                    "p g l (c w) -> p (g l) c w", c=C)
                for c in range(C):
                    nc.tensor.matmul(vps, lhsT=band_sb, rhs=rhs[:, :, c, :],
                                     start=(c == 0), stop=(c == C - 1))
                nc.scalar.copy(v_sb[:, b], vps)

            # ---- tail: horizontal 5-tap per 16-channel pair + out DMAs ----
            res_all = s.tile([M, O, NW], dt.float16)
            Epair = s.tile([M, 2, 2 * OB, W - 1], dt.float16)
            Tpair = s.tile([M, 2, 2 * OB, NW], dt.float16)
            for pr in range(2):
                vp = v_sb[:, 2 * pr : 2 * pr + 2].rearrange("m b o w -> m (b o) w")
                E = Epair[:, pr]
                nc.vector.tensor_add(E, vp[:, :, 0 : W - 1], vp[:, :, 1:W])
                T = Tpair[:, pr]
                nc.vector.tensor_add(T, E[:, :, 0:NW], E[:, :, 2 : NW + 2])
                res = res_all[:, pr * 2 * OB : (pr + 1) * 2 * OB]
                nc.vector.tensor_add(res, T, vp[:, :, 4 : 4 + NW])
                # one casting DMA per pair (fp16 -> fp32, SWDGE/gpsimd-only;
                # extra triggers would serialize ~700ns each on the queue)
                lo_ch = pr * 2 * OB
                nc.gpsimd.dma_start(out=out_d[:, lo_ch : lo_ch + 2 * OB],
                                    in_=res_all[:, lo_ch : lo_ch + 2 * OB])
    return nc


def _legalize_multiwaits(bir_json_bytes):
    """Split multi-wait instructions into standalone EventSemaphore waits.

    The walrus codegen in this toolchain accepts at most one inline sync
    wait per compute-engine instruction ("Too many sync wait commands").
    Tile emits joins with several waits; moving the extras onto
    EventSemaphore instructions issued immediately before, on the same
    engine queue, is semantically identical.
    """
    import json

    j = json.loads(bir_json_bytes)
    for fn in j["functions"]:
        for blk in fn["blocks"]:
            new_insts = []
            for inst in blk["instructions"]:
                si = inst.get("sync_info") or {}
                waits = si.get("on_wait") or []
                if len(waits) > 1:
                    for k, w in enumerate(waits[:-1]):
                        new_insts.append(
                            {
                                "debug": inst.get("debug"),
                                "engine": inst["engine"],
                                "ins": [],
                                "name": f"{inst['name']}_syncw{k}",
                                "opcode": "EventSemaphore",
                                "outs": [],
                                "sync_info": {"on_update": [], "on_wait": [w]},
                            }
                        )
                    si["on_wait"] = [waits[-1]]
                new_insts.append(inst)
            blk["instructions"] = new_insts
    return json.dumps(j).encode()


def _band_np():
    band = np.zeros((P, M), np.float16)
    for b in range(BPC):
        for h in range(H):
            for i in range(NH):
                if 0 <= h - i < KS:
                    band[b * H + h, b * NH + i] = 2.0 ** -10
    return band


def _get_built(scale, bias):
    key = (round(float(scale), 9), round(float(bias), 9))
    if key not in _CACHE:
        nc = _build(float(scale), float(bias))
        legal = _legalize_multiwaits(nc.to_json_bytes())
        nc.to_json_bytes = lambda: legal
        _CACHE[key] = nc
    return _CACHE[key]


def kernel(x, left_bounds, right_bounds):
    x = np.ascontiguousarray(x, np.float32)
    lb = np.asarray(left_bounds, np.float32).reshape(O, -1)
    rb = np.asarray(right_bounds, np.float32).reshape(O, -1)
    widths = rb[:, 0] - lb[:, 0]
    width = float(widths[0])
    # the kernel's bin decomposition requires uniform contiguous bins
    assert np.allclose(widths, width, rtol=1e-5), "non-uniform bounds unsupported"
    assert np.allclose(lb[1:, 0], rb[:-1, 0], atol=1e-6), "bins must tile the domain"
    scale = 1.0 / width
    # +64 shift keeps z positive for |x| within ~5 sigma so trunc-mod == floor-mod;
    # bin o lives at idx == 64 + o, matching the iota compare constants.
    bias = -float(lb[0, 0]) * scale + 64.0

    nc = _get_built(scale, bias)
    band = _band_np()  # [P, M] fp16
    c_lo = np.broadcast_to(
        np.arange(NLO, dtype=np.float16).reshape(1, NLO, 1), (P, NLO, FD)
    ).reshape(P, NLO * FD)
    c_hi = np.broadcast_to(
        (16 + np.arange(NHI, dtype=np.float16)).reshape(1, NHI, 1), (P, NHI, FD)
    ).reshape(P, NHI * FD)
    blob2 = np.ascontiguousarray(
        np.concatenate([band, c_lo, c_hi], axis=1)
    ).view(np.float32)
    in_maps = []
    for k in range(NCORES):
        xc = x[BPC * k : BPC * (k + 1)]  # [BPC, C, H, W]
        xt = np.ascontiguousarray(xc.transpose(0, 2, 1, 3).reshape(P, C * W))
        in_maps.append({"xb": xt, "blob2": blob2})

    from concourse.bass_utils import run_bass_kernel_spmd

    r = run_bass_kernel_spmd(nc, in_maps, list(range(NCORES)))
    global _LAST_RESULT
    _LAST_RESULT = r
    parts = []
    for k in range(NCORES):
        oc = r.results[k]["out"]  # [M, O, NW] = [(b i), o, j]
        oc = oc.reshape(BPC, NH, O, NW).transpose(0, 2, 1, 3)
        parts.append(np.ascontiguousarray(oc))
    out = np.concatenate(parts, axis=0)
    return np.ascontiguousarray(out, np.float32)


_LAST_RESULT = None
